# revision 48
# baseline (speedup 1.0000x reference)
"""Trainium2 Bass kernel for the real-space Ewald potential.

Computes  out = NORM/(4*pi) * sum_{i!=j} (q_i . q_j) * erf(|r_i-r_j|/sqrt(2)) / |r_i-r_j|

Strategy (8 NeuronCores, SPMD):
  - The N x N pair grid is split into 8x8 super-tiles of 512x512; each core
    gets 9 half-super-tiles (256x512): both halves of its own diagonal
    super-tile (units 0/1, weight 1) plus 7 upper-triangle halves (weight 2).
  - d2_ij = s_i + s_j - 2 r_i.r_j comes from ONE K=18 bf16 matmul: r and s
    are split hi/lo (hi/mid/lo for s) into bf16 on the host, so the PE runs
    at full bf16 rate while keeping |d2 err| < 2e-4.
  - Phase 1 (sqrt ACT table): d = sqrt(d2 + 5e-4) on the scalar engine;
    rinv = 1/d via the single-instruction DVE RECIPROCAL_APPROX_FAST
    (bf16 out) -- the DVE is otherwise idle in phase 1.  The pair-grid
    diagonal is zeroed on-device by affine_select on rinv for units 0/1
    (no host-side diagonal correction needed).
  - Phase 2 (erf ACT table): erf(d/sqrt(2)) in merged spans (3,3,2,1 units,
    bf16 out), then F = erf * rinv as plain tensor_tensor multiplies split
    between the vector and GPSIMD engines.
  - G[c,i] = sum_j q[j,c] F[j,i] is a K=128 bf16 matmul with q split hi/lo
    (lhsT [qh|ql], M=16); four super-tiles share a PSUM bank via PE column
    quadrants 0/32/64/96.  Each bank is finalized with a fused
    tensor_tensor_reduce against fp32 q.  Each core emits one scalar
    partial; the host sums the 8 partials and applies the constant scale.
"""

import os
import sys

import ml_dtypes
import numpy as np

for _p in ("/opt/trn_rl_repo",):
    if os.path.isdir(_p) and _p not in sys.path:
        sys.path.insert(0, _p)

import concourse.bacc as bacc  # noqa: E402
import concourse.mybir as mybir  # noqa: E402
import concourse.tile as tile  # noqa: E402
from concourse.bass_utils import run_bass_kernel_spmd  # noqa: E402
from concourse.dve_ops import (  # noqa: E402
    RECIP_APPROX_FAST_CONSTS,
    RECIPROCAL_APPROX_FAST,
)

N = 4096  # atoms
NQ = 8  # charge channels
NCORES = 8
CH = 512  # super-tile edge (i-chunk width / j-chunk height)
NU = 9  # half-super-tile units per core: 72 halves / 8 cores, exact balance
NGB = (NU + 3) // 4  # G PSUM banks (4 units per bank, PE quadrants)
BIAS = 5e-4  # sqrt(d2 + BIAS): guards bf16-split cancellation (|err| < 2e-4)
INV_SQRT2 = 0.7071067811865476
TWOPI = 2.0 * np.pi
NORM_FACTOR = 90.0474
BF16 = ml_dtypes.bfloat16


ERF_SPANS = {0: 3, 3: 3, 6: 2, 8: 1}  # merged erf instruction sizes

TRACE = bool(os.environ.get("BASS_EWALD_TRACE"))
LAST_RESULTS = None  # BassKernelResults of the most recent run (for test.py)

_prog = None


def _build_program():
    f32 = mybir.dt.float32
    bf16 = mybir.dt.bfloat16
    AF = mybir.ActivationFunctionType
    OP = mybir.AluOpType
    RC = RECIP_APPROX_FAST_CONSTS

    nc = bacc.Bacc("TRN2", target_bir_lowering=False, debug=False, num_devices=NCORES)
    at_d = nc.dram_tensor("AT", [18, NU * CH], bf16, kind="ExternalInput")
    bt_d = nc.dram_tensor("BT", [18, NU * 256], bf16, kind="ExternalInput")
    qw_d = nc.dram_tensor("QW", [128, NU * 32], bf16, kind="ExternalInput")
    qf_d = nc.dram_tensor("QF", [128, NGB * CH], bf16, kind="ExternalInput")
    out_d = nc.dram_tensor("OUT", [1, NGB], f32, kind="ExternalOutput")

    with tile.TileContext(nc) as tc:
        with (
            tc.tile_pool(name="const", bufs=1) as cp,
            tc.tile_pool(name="work", bufs=4) as wp,
            tc.tile_pool(name="single", bufs=1) as sp,
            tc.tile_pool(name="pd", bufs=2, space="PSUM") as pd,
            tc.tile_pool(name="pg", bufs=1, space="PSUM") as pg,
        ):
            at = cp.tile([18, NU * CH], bf16)
            bt = cp.tile([18, NU * 256], bf16)
            qw = cp.tile([128, NU * 32], bf16)
            qf = cp.tile([128, NGB * CH], bf16)
            # Input loads spread over four engine DMA queues; the first
            # AT/BT chunks (units 0-2) land first so the first d2 matmul
            # isn't gated on the whole load.
            for p in range(3):
                sl = slice(p * 3 * CH, (p + 1) * 3 * CH)
                nc.sync.dma_start(at[:, sl], at_d[:, sl])
            b0 = slice(0, 3 * 256)
            nc.scalar.dma_start(bt[:, b0], bt_d[:, b0])
            for p in (1, 2):
                sl2 = slice(p * 3 * 256, (p + 1) * 3 * 256)
                nc.gpsimd.dma_start(bt[:, sl2], bt_d[:, sl2])
            nc.sync.dma_start(qw[:], qw_d[:])
            nc.gpsimd.dma_start(qf[:], qf_d[:])

            dall = cp.tile([128, NU * 1024], f32)
            rall = cp.tile([128, NU * 1024], bf16)
            et = cp.tile([128, NU * 1024], bf16)
            ones = cp.tile([128, 1], f32)
            nc.vector.memset(ones[:], 1.0)
            bias_t = cp.tile([128, 1], f32)
            nc.vector.memset(bias_t[:], BIAS)
            gbanks = []
            for k in range(NGB):
                gk = pg.tile([128, CH], f32, tag=f"g{k}")
                nc.vector.memset(gk[:], 0.0)
                gbanks.append(gk)
            rtmp = sp.tile([128, 1024], bf16, tag="rtmp")

            # Phase 1: d2 matmuls + sqrt (sqrt ACT table set) + DVE recip.
            for u in range(NU):
                ps = pd.tile([128, 1024], f32, tag="d2")
                for loc in (0, 1):
                    nc.tensor.matmul(
                        ps[:, loc * CH : (loc + 1) * CH],
                        bt[:, u * 256 + loc * 128 : u * 256 + (loc + 1) * 128],
                        at[:, u * CH : (u + 1) * CH],
                        start=True,
                        stop=True,
                    )
                dsl = dall[:, u * 1024 : (u + 1) * 1024]
                nc.scalar.activation(dsl, ps[:], AF.Sqrt, bias=bias_t[:])
                rsl = rall[:, u * 1024 : (u + 1) * 1024]
                if u < 2:
                    # Diagonal half-super-tile: recip into scratch, then
                    # zero the pair-grid diagonal (j == i) via affine
                    # select: iota = -256*hh - p - 128*loc + c == 0 there.
                    nc.vector._custom_dve(
                        RECIPROCAL_APPROX_FAST,
                        out=rtmp[:],
                        in0=dsl,
                        s0=RC["s0"],
                        s1=RC["s1"],
                        imm2=RC["imm2"],
                    )
                    nc.gpsimd.affine_select(
                        rsl,
                        rtmp[:],
                        pattern=[[-128, 2], [1, 512]],
                        compare_op=OP.not_equal,
                        fill=0.0,
                        base=(0 if u == 0 else -256),
                        channel_multiplier=-1,
                    )
                else:
                    nc.vector._custom_dve(
                        RECIPROCAL_APPROX_FAST,
                        out=rsl,
                        in0=dsl,
                        s0=RC["s0"],
                        s1=RC["s1"],
                        imm2=RC["imm2"],
                    )

            # Keep the two ACT table sets in disjoint program ranges.
            tc.no_sync_barrier()

            # Phase 2: erf (merged spans) + F-multiply + G matmuls.
            # F-multiplies run on the DVE in 512-column halves so each G
            # matmul starts as soon as its half is ready (concurrent GPSIMD
            # tensor_tensor slows the DVE ~4x -- SBUF contention -- so only
            # the off-critical-path bank finalizes go to GPSIMD).
            acc = sp.tile([128, 1], f32, tag="acc")
            fscr = sp.tile([128, NGB * CH], bf16, tag="fscr")
            gsb = sp.tile([128, 2 * CH], bf16, tag="gsb")
            res = sp.tile([1, NGB], f32, tag="res")

            for u in range(NU):
                k, m = divmod(u, 4)  # G bank, quadrant
                span = ERF_SPANS.get(u)
                if span is not None:
                    nc.scalar.activation(
                        et[:, u * 1024 : (u + span) * 1024],
                        dall[:, u * 1024 : (u + span) * 1024],
                        AF.Erf,
                        scale=INV_SQRT2,
                    )
                f = wp.tile([128, 1024], bf16, tag="f")
                nc.vector.tensor_tensor(
                    f[:],
                    et[:, u * 1024 : (u + 1) * 1024],
                    rall[:, u * 1024 : (u + 1) * 1024],
                    OP.mult,
                )
                for loc in (0, 1):
                    nc.tensor.matmul(
                        gbanks[k][32 * m : 32 * m + 16, :],
                        qw[:, u * 32 + loc * 16 : u * 32 + (loc + 1) * 16],
                        f[:, loc * CH : (loc + 1) * CH],
                        start=(loc == 0),
                        stop=(loc == 1),
                        tile_position=(0, 32 * m),
                    )

            # Bank finalizes: ACT (idle after the erf chain) copies each G
            # bank PSUM->SBUF, then the DVE contracts against fp32 q.  The
            # fused tensor_tensor_reduce is fine with SBUF operands (its
            # PSUM-input path crashes the exec unit on this HW).
            # Bank finalizes.  (tensor_tensor_reduce would fuse mult+reduce
            # but crashes the exec unit on this HW even with SBUF operands.)
            # Banks 0/1 (ready early): ACT copies PSUM->SBUF bf16, DVE does
            # the 2x bf16 multiply, GPSIMD does the all-axis partition sum
            # straight to a scalar.  Bank 2 (last): DVE multiply direct from
            # PSUM (skips the copy latency), DVE free-axis reduce, and the
            # partition sum rides the final matmul.  Host adds 3 scalars.
            for kk in (0, 1):
                gs = gsb[:, kk * CH : (kk + 1) * CH]
                fs = fscr[:, kk * CH : (kk + 1) * CH]
                nc.scalar.copy(gs, gbanks[kk][:])
                nc.vector.tensor_tensor(
                    fs, gs, qf[:, kk * CH : (kk + 1) * CH], OP.mult
                )
                with nc.allow_low_precision(reason="bank partial; rel err budget 2e-2"):
                    nc.gpsimd.reduce_sum(
                        res[:, kk + 1 : kk + 2], fs, axis=mybir.AxisListType.XYZWC
                    )
            fs2 = fscr[:, 2 * CH : 3 * CH]
            nc.vector.tensor_tensor(fs2, gbanks[2][:], qf[:, 2 * CH : 3 * CH], OP.mult)
            nc.vector.reduce_sum(acc[:], fs2, axis=mybir.AxisListType.X)
            tot = pg.tile([1, 1], f32, tag="tot")
            nc.tensor.matmul(tot[:], acc[:], ones[:], start=True, stop=True)
            nc.scalar.copy(res[:, 0:1], tot[:])
            nc.sync.dma_start(out_d[:], res[:])

    nc.compile()
    return nc


def _get_program():
    global _prog
    if _prog is None:
        _prog = _build_program()
    return _prog


def _bf16_split(x32, parts):
    """Split fp32 array into `parts` bf16 arrays summing to x32 (greedy)."""
    out = []
    rem = x32.astype(np.float64)
    for _ in range(parts):
        p = rem.astype(np.float32).astype(BF16)
        out.append(p)
        rem = rem - p.astype(np.float64)
    return out


def _host_prep(q, r):
    q = np.ascontiguousarray(np.asarray(q, np.float32))
    r = np.ascontiguousarray(np.asarray(r, np.float32))
    r64 = r.astype(np.float64)
    s64 = (r64 * r64).sum(1)

    rh, rl = _bf16_split(r, 2)  # [N,3] bf16 each
    m2rh, m2rl = (-2.0 * rh.astype(np.float32)).astype(BF16), (
        -2.0 * rl.astype(np.float32)
    ).astype(BF16)
    sh, sm, sl = _bf16_split(s64, 3)  # [N] bf16 each
    onesN = np.ones(N, BF16)

    # rhs rows (i side) pair with lhsT rows (j side), K=18:
    #   -2rh_j*rh_i, -2rh_j*rl_i, -2rl_j*rh_i, -2rl_j*rl_i (12 rows),
    #   (sh+sm+sl)_j * 1 (3 rows), 1 * (sh+sm+sl)_i (3 rows)
    A18 = np.concatenate(
        [rh.T, rl.T, rh.T, rl.T, [onesN, onesN, onesN], [sh, sm, sl]]
    ).astype(BF16)  # [18, N]
    B18 = np.concatenate(
        [m2rh.T, m2rh.T, m2rl.T, m2rl.T, [sh, sm, sl], [onesN, onesN, onesN]]
    ).astype(BF16)  # [18, N]

    qT = np.ascontiguousarray(q.T)  # [NQ, N] f32

    # Unit deal: core c takes both halves of its own diagonal super-tile
    # (units 0/1, so the on-device diagonal mask sees hh=0 then hh=1 in a
    # fixed program position), then 7 upper-triangle halves round-robin.
    offd = [
        (a, b, hh, 2.0) for a in range(8) for b in range(a + 1, 8) for hh in (0, 1)
    ]
    assignments = []
    for c in range(NCORES):
        assignments.append(
            [(c, c, 0, 1.0), (c, c, 1, 1.0)] + offd[c::NCORES]
        )

    in_maps = []
    for c in range(NCORES):
        AT = np.empty((18, NU * CH), BF16)
        BT = np.empty((18, NU * 256), BF16)
        QW = np.empty((128, NU * 32), BF16)
        QF = np.zeros((128, NGB * CH), BF16)
        for u, (a, b, hh, w) in enumerate(assignments[c]):
            k, m = divmod(u, 4)
            AT[:, u * CH : (u + 1) * CH] = A18[:, b * CH : (b + 1) * CH]
            BT[:, u * 256 : (u + 1) * 256] = B18[
                :, a * CH + hh * 256 : a * CH + (hh + 1) * 256
            ]
            # Finalize reads quadrant rows 32m + [0..16): both the qh and ql
            # halves of G contract against the same fp32 qT chunk.
            QF[32 * m : 32 * m + NQ, k * CH : (k + 1) * CH] = qT[
                :, b * CH : (b + 1) * CH
            ]
            QF[32 * m + NQ : 32 * m + 2 * NQ, k * CH : (k + 1) * CH] = qT[
                :, b * CH : (b + 1) * CH
            ]
            wq = (
                w * q[a * CH + hh * 256 : a * CH + (hh + 1) * 256, :]
            ).astype(np.float32)  # [256, NQ]
            wqh, wql = _bf16_split(wq, 2)
            blk = np.concatenate([wqh, wql], axis=1)  # [256, 16]
            QW[:, u * 32 : (u + 1) * 32] = (
                blk.reshape(2, 128, 2 * NQ).transpose(1, 0, 2).reshape(128, 32)
            )
        in_maps.append({"AT": AT, "BT": BT, "QW": QW, "QF": QF})
    return in_maps


def kernel(q, r, cell):
    global LAST_RESULTS
    in_maps = _host_prep(q, r)
    nc = _get_program()
    res = run_bass_kernel_spmd(nc, in_maps, list(range(NCORES)), trace=TRACE)
    LAST_RESULTS = res
    S = sum(float(res.results[c]["OUT"].sum()) for c in range(NCORES))
    val = S / TWOPI / 2.0 * NORM_FACTOR
    return np.array([val], np.float32)


# revision 51
# speedup vs baseline: 1.2060x; 1.2060x over previous
"""Trainium2 Bass kernel for the real-space Ewald potential.

Computes  out = NORM/(4*pi) * sum_{i!=j} (q_i . q_j) * erf(|r_i-r_j|/sqrt(2)) / |r_i-r_j|

Strategy (8 NeuronCores, SPMD):
  - The N x N pair grid is split into 8x8 super-tiles of 512x512; each core
    gets 9 half-super-tiles (256x512): both halves of its own diagonal
    super-tile (units 0/1, weight 1) plus 7 upper-triangle halves (weight 2).
  - d2_ij = s_i + s_j - 2 r_i.r_j comes from ONE K=18 bf16 matmul: r and s
    are split hi/lo (hi/mid/lo for s) into bf16 on the host, so the PE runs
    at full bf16 rate while keeping |d2 err| < 2e-4.
  - Phase 1 (sqrt ACT table): d = sqrt(d2 + 5e-4) on the scalar engine;
    rinv = 1/d via the single-instruction DVE RECIPROCAL_APPROX_FAST
    (bf16 out) -- the DVE is otherwise idle in phase 1.  The pair-grid
    diagonal is zeroed on-device by affine_select on rinv for units 0/1
    (no host-side diagonal correction needed).
  - Phase 2 (erf ACT table): erf(d/sqrt(2)) in merged spans (3,3,2,1 units,
    bf16 out), then F = erf * rinv as plain tensor_tensor multiplies split
    between the vector and GPSIMD engines.
  - G[c,i] = sum_j q[j,c] F[j,i] is a K=128 bf16 matmul with q split hi/lo
    (lhsT [qh|ql], M=16); four super-tiles share a PSUM bank via PE column
    quadrants 0/32/64/96.  Each bank is finalized with a fused
    tensor_tensor_reduce against fp32 q.  Each core emits one scalar
    partial; the host sums the 8 partials and applies the constant scale.
"""

import os
import sys

import ml_dtypes
import numpy as np

for _p in ("/opt/trn_rl_repo",):
    if os.path.isdir(_p) and _p not in sys.path:
        sys.path.insert(0, _p)

import concourse.bacc as bacc  # noqa: E402
import concourse.mybir as mybir  # noqa: E402
import concourse.tile as tile  # noqa: E402
from concourse.bass_utils import run_bass_kernel_spmd  # noqa: E402
from concourse.dve_ops import (  # noqa: E402
    RECIP_APPROX_FAST_CONSTS,
    RECIPROCAL_APPROX_FAST,
)

N = 4096  # atoms
NQ = 8  # charge channels
NCORES = 8
CH = 512  # super-tile edge (i-chunk width / j-chunk height)
NU = 9  # half-super-tile units per core: 72 halves / 8 cores, exact balance
NGB = (NU + 3) // 4  # G PSUM banks (4 units per bank, PE quadrants)
BIAS = 5e-4  # sqrt(d2 + BIAS): guards bf16-split cancellation (|err| < 2e-4)
INV_SQRT2 = 0.7071067811865476
TWOPI = 2.0 * np.pi
NORM_FACTOR = 90.0474
BF16 = ml_dtypes.bfloat16


ERF_SPANS = {0: 3, 3: 3, 6: 2, 8: 1}  # merged erf instruction sizes

TRACE = bool(os.environ.get("BASS_EWALD_TRACE"))
LAST_RESULTS = None  # BassKernelResults of the most recent run (for test.py)

_prog = None


def _build_program():
    f32 = mybir.dt.float32
    bf16 = mybir.dt.bfloat16
    AF = mybir.ActivationFunctionType
    OP = mybir.AluOpType
    RC = RECIP_APPROX_FAST_CONSTS

    nc = bacc.Bacc("TRN2", target_bir_lowering=False, debug=False, num_devices=NCORES)
    at_d = nc.dram_tensor("AT", [18, NU * CH], bf16, kind="ExternalInput")
    bt_d = nc.dram_tensor("BT", [18, NU * 256], bf16, kind="ExternalInput")
    qw_d = nc.dram_tensor("QW", [128, NU * 32], bf16, kind="ExternalInput")
    qf_d = nc.dram_tensor("QF", [128, NGB * CH], bf16, kind="ExternalInput")
    out_d = nc.dram_tensor("OUT", [NGB, 1], f32, kind="ExternalOutput")

    with tile.TileContext(nc) as tc:
        with (
            tc.tile_pool(name="const", bufs=1) as cp,
            tc.tile_pool(name="work", bufs=4) as wp,
            tc.tile_pool(name="single", bufs=1) as sp,
            tc.tile_pool(name="pd", bufs=2, space="PSUM") as pd,
            tc.tile_pool(name="pg", bufs=1, space="PSUM") as pg,
        ):
            at = cp.tile([18, NU * CH], bf16)
            bt = cp.tile([18, NU * 256], bf16)
            qw = cp.tile([128, NU * 32], bf16)
            qf = cp.tile([128, NGB * CH], bf16)
            # Input loads spread over four engine DMA queues; the first
            # AT/BT chunks (units 0-2) land first so the first d2 matmul
            # isn't gated on the whole load.
            for p in range(3):
                sl = slice(p * 3 * CH, (p + 1) * 3 * CH)
                nc.sync.dma_start(at[:, sl], at_d[:, sl])
            b0 = slice(0, 3 * 256)
            nc.scalar.dma_start(bt[:, b0], bt_d[:, b0])
            for p in (1, 2):
                sl2 = slice(p * 3 * 256, (p + 1) * 3 * 256)
                nc.gpsimd.dma_start(bt[:, sl2], bt_d[:, sl2])
            nc.sync.dma_start(qw[:], qw_d[:])
            nc.gpsimd.dma_start(qf[:], qf_d[:])

            dall = cp.tile([128, NU * 1024], f32)
            rall = cp.tile([128, NU * 1024], bf16)
            et = cp.tile([128, NU * 1024], bf16)
            ones = cp.tile([128, 1], f32)
            nc.vector.memset(ones[:], 1.0)
            bias_t = cp.tile([128, 1], f32)
            nc.vector.memset(bias_t[:], BIAS)
            gbanks = []
            for k in range(NGB):
                gk = pg.tile([128, CH], f32, tag=f"g{k}")
                nc.vector.memset(gk[:], 0.0)
                gbanks.append(gk)
            rtmp = sp.tile([128, 1024], bf16, tag="rtmp")

            # Phase 1: d2 matmuls + sqrt (sqrt ACT table set) + DVE recip.
            for u in range(NU):
                ps = pd.tile([128, 1024], f32, tag="d2")
                for loc in (0, 1):
                    nc.tensor.matmul(
                        ps[:, loc * CH : (loc + 1) * CH],
                        bt[:, u * 256 + loc * 128 : u * 256 + (loc + 1) * 128],
                        at[:, u * CH : (u + 1) * CH],
                        start=True,
                        stop=True,
                    )
                dsl = dall[:, u * 1024 : (u + 1) * 1024]
                nc.scalar.activation(dsl, ps[:], AF.Sqrt, bias=bias_t[:])
                rsl = rall[:, u * 1024 : (u + 1) * 1024]
                if u < 2:
                    # Diagonal half-super-tile: recip into scratch, then
                    # zero the pair-grid diagonal (j == i) via affine
                    # select: iota = -256*hh - p - 128*loc + c == 0 there.
                    nc.vector._custom_dve(
                        RECIPROCAL_APPROX_FAST,
                        out=rtmp[:],
                        in0=dsl,
                        s0=RC["s0"],
                        s1=RC["s1"],
                        imm2=RC["imm2"],
                    )
                    nc.gpsimd.affine_select(
                        rsl,
                        rtmp[:],
                        pattern=[[-128, 2], [1, 512]],
                        compare_op=OP.not_equal,
                        fill=0.0,
                        base=(0 if u == 0 else -256),
                        channel_multiplier=-1,
                    )
                else:
                    nc.vector._custom_dve(
                        RECIPROCAL_APPROX_FAST,
                        out=rsl,
                        in0=dsl,
                        s0=RC["s0"],
                        s1=RC["s1"],
                        imm2=RC["imm2"],
                    )

            # Keep the two ACT table sets in disjoint program ranges.
            tc.no_sync_barrier()

            # Phase 2: erf (merged spans) + F-multiply + G matmuls.
            # F-multiplies run on the DVE in 512-column halves so each G
            # matmul starts as soon as its half is ready (concurrent GPSIMD
            # tensor_tensor slows the DVE ~4x -- SBUF contention -- so only
            # the off-critical-path bank finalizes go to GPSIMD).
            acc = sp.tile([128, NGB], f32, tag="acc")
            fscr = sp.tile([128, NGB * CH], bf16, tag="fscr")
            gsb = sp.tile([128, 2 * CH], bf16, tag="gsb")

            for u in range(NU):
                k, m = divmod(u, 4)  # G bank, quadrant
                span = ERF_SPANS.get(u)
                if span is not None:
                    nc.scalar.activation(
                        et[:, u * 1024 : (u + span) * 1024],
                        dall[:, u * 1024 : (u + span) * 1024],
                        AF.Erf,
                        scale=INV_SQRT2,
                    )
                f = wp.tile([128, 1024], bf16, tag="f")
                nc.vector.tensor_tensor(
                    f[:],
                    et[:, u * 1024 : (u + 1) * 1024],
                    rall[:, u * 1024 : (u + 1) * 1024],
                    OP.mult,
                )
                for loc in (0, 1):
                    nc.tensor.matmul(
                        gbanks[k][32 * m : 32 * m + 16, :],
                        qw[:, u * 32 + loc * 16 : u * 32 + (loc + 1) * 16],
                        f[:, loc * CH : (loc + 1) * CH],
                        start=(loc == 0),
                        stop=(loc == 1),
                        tile_position=(0, 32 * m),
                    )

            # Bank finalizes: ACT (idle after the erf chain) copies each G
            # bank PSUM->SBUF, then the DVE contracts against fp32 q.  The
            # fused tensor_tensor_reduce is fine with SBUF operands (its
            # PSUM-input path crashes the exec unit on this HW).
            # Bank finalizes.  (tensor_tensor_reduce would fuse mult+reduce
            # but crashes the exec unit on this HW even with SBUF operands;
            # GPSIMD's all-axis reduce takes 2.5us -- too slow.)  Banks 0/1
            # (ready early): ACT copies PSUM->SBUF bf16 so the DVE multiply
            # runs at the 2x bf16 rate.  Bank 2 (completes last): multiply
            # direct from PSUM, skipping the copy latency.  Partition sums
            # of all three bank columns ride one matmul; host adds 3 values.
            for kk in (0, 1):
                gs = gsb[:, kk * CH : (kk + 1) * CH]
                fs = fscr[:, kk * CH : (kk + 1) * CH]
                nc.scalar.copy(gs, gbanks[kk][:])
                nc.vector.tensor_tensor(
                    fs, gs, qf[:, kk * CH : (kk + 1) * CH], OP.mult
                )
                nc.vector.reduce_sum(
                    acc[:, kk : kk + 1], fs, axis=mybir.AxisListType.X
                )
            fs2 = fscr[:, 2 * CH : 3 * CH]
            nc.vector.tensor_tensor(fs2, gbanks[2][:], qf[:, 2 * CH : 3 * CH], OP.mult)
            nc.vector.reduce_sum(acc[:, 2:3], fs2, axis=mybir.AxisListType.X)
            tot = pg.tile([NGB, 1], f32, tag="tot")
            nc.tensor.matmul(tot[:], acc[:], ones[:], start=True, stop=True)
            res = sp.tile([NGB, 1], f32, tag="res")
            nc.scalar.copy(res[:], tot[:])
            nc.sync.dma_start(out_d[:], res[:])

    nc.compile()
    return nc


def _get_program():
    global _prog
    if _prog is None:
        _prog = _build_program()
    return _prog


def _bf16_split(x32, parts):
    """Split fp32 array into `parts` bf16 arrays summing to x32 (greedy)."""
    out = []
    rem = x32.astype(np.float64)
    for _ in range(parts):
        p = rem.astype(np.float32).astype(BF16)
        out.append(p)
        rem = rem - p.astype(np.float64)
    return out


def _host_prep(q, r):
    q = np.ascontiguousarray(np.asarray(q, np.float32))
    r = np.ascontiguousarray(np.asarray(r, np.float32))
    r64 = r.astype(np.float64)
    s64 = (r64 * r64).sum(1)

    rh, rl = _bf16_split(r, 2)  # [N,3] bf16 each
    m2rh, m2rl = (-2.0 * rh.astype(np.float32)).astype(BF16), (
        -2.0 * rl.astype(np.float32)
    ).astype(BF16)
    sh, sm, sl = _bf16_split(s64, 3)  # [N] bf16 each
    onesN = np.ones(N, BF16)

    # rhs rows (i side) pair with lhsT rows (j side), K=18:
    #   -2rh_j*rh_i, -2rh_j*rl_i, -2rl_j*rh_i, -2rl_j*rl_i (12 rows),
    #   (sh+sm+sl)_j * 1 (3 rows), 1 * (sh+sm+sl)_i (3 rows)
    A18 = np.concatenate(
        [rh.T, rl.T, rh.T, rl.T, [onesN, onesN, onesN], [sh, sm, sl]]
    ).astype(BF16)  # [18, N]
    B18 = np.concatenate(
        [m2rh.T, m2rh.T, m2rl.T, m2rl.T, [sh, sm, sl], [onesN, onesN, onesN]]
    ).astype(BF16)  # [18, N]

    qT = np.ascontiguousarray(q.T)  # [NQ, N] f32

    # Unit deal: core c takes both halves of its own diagonal super-tile
    # (units 0/1, so the on-device diagonal mask sees hh=0 then hh=1 in a
    # fixed program position), then 7 upper-triangle halves round-robin.
    offd = [
        (a, b, hh, 2.0) for a in range(8) for b in range(a + 1, 8) for hh in (0, 1)
    ]
    assignments = []
    for c in range(NCORES):
        assignments.append(
            [(c, c, 0, 1.0), (c, c, 1, 1.0)] + offd[c::NCORES]
        )

    in_maps = []
    for c in range(NCORES):
        AT = np.empty((18, NU * CH), BF16)
        BT = np.empty((18, NU * 256), BF16)
        QW = np.empty((128, NU * 32), BF16)
        QF = np.zeros((128, NGB * CH), BF16)
        for u, (a, b, hh, w) in enumerate(assignments[c]):
            k, m = divmod(u, 4)
            AT[:, u * CH : (u + 1) * CH] = A18[:, b * CH : (b + 1) * CH]
            BT[:, u * 256 : (u + 1) * 256] = B18[
                :, a * CH + hh * 256 : a * CH + (hh + 1) * 256
            ]
            # Finalize reads quadrant rows 32m + [0..16): both the qh and ql
            # halves of G contract against the same fp32 qT chunk.
            QF[32 * m : 32 * m + NQ, k * CH : (k + 1) * CH] = qT[
                :, b * CH : (b + 1) * CH
            ]
            QF[32 * m + NQ : 32 * m + 2 * NQ, k * CH : (k + 1) * CH] = qT[
                :, b * CH : (b + 1) * CH
            ]
            wq = (
                w * q[a * CH + hh * 256 : a * CH + (hh + 1) * 256, :]
            ).astype(np.float32)  # [256, NQ]
            wqh, wql = _bf16_split(wq, 2)
            blk = np.concatenate([wqh, wql], axis=1)  # [256, 16]
            QW[:, u * 32 : (u + 1) * 32] = (
                blk.reshape(2, 128, 2 * NQ).transpose(1, 0, 2).reshape(128, 32)
            )
        in_maps.append({"AT": AT, "BT": BT, "QW": QW, "QF": QF})
    return in_maps


def kernel(q, r, cell):
    global LAST_RESULTS
    in_maps = _host_prep(q, r)
    nc = _get_program()
    res = run_bass_kernel_spmd(nc, in_maps, list(range(NCORES)), trace=TRACE)
    LAST_RESULTS = res
    S = sum(float(res.results[c]["OUT"].sum()) for c in range(NCORES))
    val = S / TWOPI / 2.0 * NORM_FACTOR
    return np.array([val], np.float32)
